# revision 37
# baseline (speedup 1.0000x reference)
"""Circular relative-position attention on 8 trn2 NeuronCores (v2).

Algorithm (per (batch,head), S=1024, hd=64):
  scores[q,k] = dot(Q[q],K[k])/8 + dot(Q[q], Wk[(k-q)%S]),  Wk[u] = rel_pos_k[min(u,S-u)]
  attn = softmax_k(scores)
  out[q] = sum_k attn[q,k]*V[k] + sum_u attn[q,(q+u)%S]*Wv[u],  Wv[u] = rel_pos_v[min(u,S-u)]

v2 layout choices (DMA-volume- and tensor-balanced):
  - all PE matmuls in bf16 (Q,K,Wk,V,Wv host-cast; Wk pre-scaled by 8 so the
    ACT exp scale 0.125 gives exp(QK/8 + rel)).
  - rel skew RS = Q@Wk^T computed natively in (q,u); sheared to (q,k) via the
    diagonal DRAM band buffer (write diag, read diag; baseline scheme).
  - rel-add into the scores PSUM on the DVE (tensor_add), not the PE.
  - exp WITHOUT normalization (pexp); softmax denominator accumulates for
    free in the output matmul via a ones-column appended to V (psO row 64).
  - V-term transpose: PE is_transpose matmuls of pexp tiles straight from
    SBUF into a bf16 PSUM tile, evicted by DVE (zero DMA for this stream).
  - rel_v-term: pexp written once to DRAM duplicated 2x wide (attq); xbar
    skew-transpose DMAs (scalar queue, critical section + real semaphore)
    give pexp_skew^T tiles.
  - normalize at the end: denom row -> reciprocal -> ones-matmul broadcast
    into rows 64:128 of the same PSUM bank -> DVE multiply -> DMA out^T.
  - software pipelining: rel_v matmuls + finalize of head bh are emitted
    during head bh+1, so the xbar latency hides under the next head's work.
  - batch is sharded across the 8 cores (1 batch each, 16 heads).
"""

import os
import sys
import numpy as np

for _p in ("/opt/trn_rl_repo", "/root/.axon_site/_ro/trn_rl_repo"):
    if os.path.isdir(_p) and _p not in sys.path:
        sys.path.insert(0, _p)

import ml_dtypes
from contextlib import ExitStack

import concourse.bass as bass
import concourse.tile as tile
from concourse import bacc, mybir
from concourse.masks import make_identity
from concourse.tile import add_dep_helper

FP32 = mybir.dt.float32
BF16 = mybir.dt.bfloat16
FP16 = mybir.dt.float16

B, S, D, H = 8, 1024, 1024, 16
HD = D // H
NCORES = 8


def build_module(nbh=H, s=S, hd=HD):
    nt = s // 128            # q/k/u tiles
    w1 = s + 256             # rel shear band buffer width
    ch = 512                 # matmul free-dim chunk (one PSUM bank of fp32)
    nc = bacc.Bacc("TRN2", target_bir_lowering=False, debug=False)

    qT = nc.dram_tensor("qT", [nbh, hd, s], BF16, kind="ExternalInput")
    kT = nc.dram_tensor("kT", [nbh, hd, s], BF16, kind="ExternalInput")
    v1 = nc.dram_tensor("v1", [nbh, 128, nt * 65], BF16, kind="ExternalInput")
    wT = nc.dram_tensor("wT", [hd, s], BF16, kind="ExternalInput")
    wv1 = nc.dram_tensor("wv1", [128, nt * 65], BF16, kind="ExternalInput")
    outT = nc.dram_tensor("outT", [nbh, hd + 1, s], FP32, kind="ExternalOutput")

    relbuf = [
        [nc.dram_tensor(f"relbuf_{pp}_{t}", [128, w1], BF16, kind="Internal")
         for t in range(nt)]
        for pp in range(nbh)
    ]
    attq = [nc.dram_tensor(f"attq_{pp}", [s, 2 * s], BF16, kind="Internal")
            for pp in range(nbh)]

    def dap(tensor, offset, pattern):
        return bass.AP(tensor, offset, pattern)

    with tile.TileContext(nc) as tc, ExitStack() as ctx:
        const_pool = ctx.enter_context(tc.tile_pool(name="const", bufs=1))
        qkv_pool = ctx.enter_context(tc.tile_pool(name="qkv", bufs=2))
        pexp_pool = ctx.enter_context(tc.tile_pool(name="pexp", bufs=10))
        att_pool = ctx.enter_context(tc.tile_pool(name="attT", bufs=9))
        dst_pool = ctx.enter_context(tc.tile_pool(name="dst", bufs=25))
        sm_pool = ctx.enter_context(tc.tile_pool(name="small", bufs=6))
        rb_pool = ctx.enter_context(tc.tile_pool(name="rb", bufs=17))
        sc_pool = ctx.enter_context(tc.tile_pool(name="sc", bufs=14))
        relb_pool = ctx.enter_context(tc.tile_pool(name="relb", bufs=8))
        ps_ab = ctx.enter_context(tc.tile_pool(name="psab", bufs=2, space="PSUM"))
        ps_tp = ctx.enter_context(tc.tile_pool(name="pstp", bufs=2, space="PSUM"))
        ps_out = ctx.enter_context(tc.tile_pool(name="psout", bufs=4, space="PSUM"))

        ident = const_pool.tile([128, 128], BF16)
        make_identity(nc, ident[:])
        wT_sb = const_pool.tile([hd, s], BF16)
        nc.sync.dma_start(wT_sb[:], wT.ap())
        wv1_sb = const_pool.tile([128, nt * 65], BF16)
        nc.sync.dma_start(wv1_sb[:], wv1.ap())

        tp_sem = nc.alloc_semaphore("tp_sem")

        def finish(prev):
            """rel_v accumulation (fresh PSUM group) + add the V-term partial
            + store (unnormalized + denom row). Host divides rows 0:64 by
            row 64 during the unshard step."""
            fbh, foutvs, fdsts = prev
            for h in range(2):
                psR = ps_out.tile([128, ch], FP32, tag="out")
                for m in range(nt):
                    nc.tensor.matmul(
                        psR[0:65, :], wv1_sb[:, m * 65:(m + 1) * 65],
                        fdsts[m][:, h * ch:(h + 1) * ch],
                        start=(m == 0), stop=(m == nt - 1))
                outsb = sm_pool.tile([65, ch], FP32, tag="outsb")
                nc.vector.tensor_add(outsb[:], psR[0:65, :], foutvs[h][:])
                nc.sync.dma_start(
                    dap(outT, fbh * (hd + 1) * s + h * ch, [[s, hd + 1], [1, ch]]),
                    outsb[:])

        def stage_load_A(bh):
            """Input loads + Phase A (RS skew matmul, diag write) + shear
            readback issue for head bh. Emitted one iteration ahead so the
            sync queue has everything in flight before the xbar critical
            section of the previous head blocks it."""
            pp = bh
            qT_sb = qkv_pool.tile([hd, s], BF16, tag="qT")
            nc.gpsimd.dma_start(qT_sb[:], qT.ap()[bh])
            kT_sb = qkv_pool.tile([hd, s], BF16, tag="kT")
            nc.gpsimd.dma_start(kT_sb[:], kT.ap()[bh])
            v1_sb = qkv_pool.tile([128, nt * 65], BF16, tag="v1")
            nc.gpsimd.dma_start(v1_sb[:], v1.ap()[bh])

            for t in range(nt):
                lhs = qT_sb[:, t * 128:(t + 1) * 128]
                relb = relb_pool.tile([128, s + 128], BF16, tag="relb")
                for h0 in range(0, s, ch):
                    psA = ps_ab.tile([128, ch], FP32, tag="big")
                    nc.tensor.matmul(
                        psA[:], lhs, wT_sb[:, h0:h0 + ch],
                        start=True, stop=True)
                    nc.vector.tensor_copy(relb[:, h0:h0 + ch], psA[:])
                # wrap band duplicates cols [0, 128)
                nc.vector.tensor_copy(relb[:, s:s + 128], relb[:, 0:128])
                # diagonal write: buf[i, i+u'] = relb[i, u'], u' in [0, s+128)
                nc.gpsimd.dma_start(
                    dap(relbuf[pp][t], 0, [[w1 + 1, 128], [1, s + 128]]),
                    relb[:])

            # shear read back: rb[i, k] = RS[q0+i, (k - q0 - i) % s]
            rbs = []
            for t in range(nt):
                q0 = t * 128
                rb = rb_pool.tile([128, s], BF16, tag="rb")
                nc.gpsimd.dma_start(
                    rb[:, 0:q0 + 128],
                    dap(relbuf[pp][t], s - q0, [[w1, 128], [1, q0 + 128]]))
                if q0 + 128 < s:
                    nc.gpsimd.dma_start(
                        rb[:, q0 + 128:s],
                        dap(relbuf[pp][t], 128, [[w1, 128], [1, s - q0 - 128]]))
                rbs.append(rb)
            return (qT_sb, kT_sb, v1_sb, rbs)

        prev1 = None
        prev2 = None
        cur = stage_load_A(0)
        for bh in range(nbh):
            pp = bh
            qT_sb, kT_sb, v1_sb, rbs = cur

            # ---- Phase B: scores (QK) + DVE rel add + exp -> pexp ----
            pexps = []
            for t in range(nt):
                q0 = t * 128
                lhs = qT_sb[:, q0:q0 + 128]
                rb = rbs[t]
                pexp = pexp_pool.tile([128, s], BF16, tag="pexp")
                for h0 in range(0, s, ch):
                    psB = ps_ab.tile([128, ch], FP32, tag="big")
                    nc.tensor.matmul(
                        psB[:], lhs, kT_sb[:, h0:h0 + ch],
                        start=True, stop=True)
                    sc = sc_pool.tile([128, ch], FP16, tag="sc")
                    nc.vector.tensor_add(sc[:], psB[:], rb[:, h0:h0 + ch])
                    nc.scalar.activation(
                        pexp[:, h0:h0 + ch], sc[:],
                        mybir.ActivationFunctionType.Exp, scale=0.125)
                pexps.append(pexp)
                # attq write: row q duplicated twice side by side (2s wide)
                nc.sync.dma_start(
                    dap(attq[pp], q0 * 2 * s, [[2 * s, 128], [1, 2 * s]]),
                    bass.AP(pexp[:].tensor, pexp[:].offset,
                            [[s, 128], [0, 2], [1, s]]))

            # ---- Phase C1: PE transposes + V matmuls (no DMA) ----
            attTs = []
            for j in range(nt):
                psT = ps_tp.tile([128, s], BF16, tag="tp")
                for t in range(nt):
                    nc.tensor.transpose(
                        psT[:, t * 128:(t + 1) * 128],
                        pexps[t][:, j * 128:(j + 1) * 128],
                        ident[:])
                attT = att_pool.tile([128, s], BF16, tag="attT")
                nc.vector.tensor_copy(attT[:], psT[:])
                attTs.append(attT)
            outvs = []
            for h in range(2):
                psO = ps_out.tile([128, ch], FP32, tag="out")
                for j in range(nt):
                    nc.tensor.matmul(
                        psO[0:65, :], v1_sb[:, j * 65:(j + 1) * 65],
                        attTs[j][:, h * ch:(h + 1) * ch],
                        start=(j == 0), stop=(j == nt - 1))
                # evict immediately: frees the PSUM bank for deeper pipelining
                outv = sm_pool.tile([65, ch], FP32, tag="outv")
                nc.vector.tensor_copy(outv[:], psO[0:65, :])
                outvs.append(outv)

            # ---- prefetch next head: loads + A + shear reads ----
            if bh + 1 < nbh:
                cur = stage_load_A(bh + 1)

            # ---- rel_v + finalize for head bh-2 (depth-3 pipeline) ----
            if prev2 is not None:
                finish(prev2)

            # ---- Phase C2: xbar skew transposes (critical section) ----
            dsts = []
            with tc.tile_critical():
                for m in range(nt):
                    dst = dst_pool.tile([128, s], BF16, tag="dst")
                    nc.scalar.dma_start_transpose(
                        dst[:], dap(attq[pp], m * 128, [[2 * s + 1, s], [1, 128]])
                    ).then_inc(tp_sem, 16)
                    dsts.append(dst)
                # wait on gpsimd so the scalar queue (next head's exps)
                # drains quickly
                nc.gpsimd.wait_ge(tp_sem, 16 * nt * (bh + 1))
            prev2 = prev1
            prev1 = (bh, outvs, dsts)

        finish(prev2)
        finish(prev1)

    nc.compile()
    return nc


_NC_CACHE = {}


def _get_module(nbh, s, hd):
    key = (nbh, s, hd)
    if key not in _NC_CACHE:
        _NC_CACHE[key] = build_module(nbh, s, hd)
    return _NC_CACHE[key]


def _prep_core_inputs(query, key, value, rel_pos_k, rel_pos_v):
    """Host-side shard + layout prep."""
    u = np.arange(S)
    g = np.minimum(u, S - u)
    wT = (rel_pos_k[g] * 8.0).T.astype(ml_dtypes.bfloat16).copy()   # (hd, S)
    wv = rel_pos_v[g].astype(ml_dtypes.bfloat16)                    # (S, hd)
    nt = S // 128
    wv1 = np.zeros((128, nt * 65), dtype=ml_dtypes.bfloat16)
    for m in range(nt):
        wv1[:, m * 65:m * 65 + 64] = wv[m * 128:(m + 1) * 128]

    in_maps = []
    for c in range(NCORES):
        q_c = query[c].reshape(S, H, HD)
        k_c = key[c].reshape(S, H, HD)
        v_c = value[c].reshape(S, H, HD)
        # (H, hd, S)
        qTc = np.ascontiguousarray(q_c.transpose(1, 2, 0)).astype(ml_dtypes.bfloat16)
        kTc = np.ascontiguousarray(k_c.transpose(1, 2, 0)).astype(ml_dtypes.bfloat16)
        # v1: (H, 128, nt*65), per k-tile j cols [65j, 65j+64) = V tile, col 64 = 1
        v_t = v_c.transpose(1, 0, 2).astype(ml_dtypes.bfloat16)     # (H, S, hd)
        v1 = np.zeros((H, 128, nt * 65), dtype=ml_dtypes.bfloat16)
        for j in range(nt):
            v1[:, :, j * 65:j * 65 + 64] = v_t[:, j * 128:(j + 1) * 128, :]
        v1[:, :, np.arange(nt) * 65 + 64] = 1.0
        in_maps.append({
            "qT": qTc, "kT": kTc, "v1": v1, "wT": wT, "wv1": wv1,
        })
    return in_maps


def kernel(**inputs):
    from concourse.bass_utils import run_bass_kernel_spmd

    query = np.asarray(inputs["query"], dtype=np.float32)
    key = np.asarray(inputs["key"], dtype=np.float32)
    value = np.asarray(inputs["value"], dtype=np.float32)
    rel_pos_k = np.asarray(inputs["rel_pos_k"], dtype=np.float32)
    rel_pos_v = np.asarray(inputs["rel_pos_v"], dtype=np.float32)

    nc = _get_module(H, S, HD)
    in_maps = _prep_core_inputs(query, key, value, rel_pos_k, rel_pos_v)
    res = run_bass_kernel_spmd(nc, in_maps, core_ids=list(range(NCORES)))

    out = np.empty((B, S, D), dtype=np.float32)
    for c in range(NCORES):
        outT = res.results[c]["outT"]                 # (H, HD+1, S)
        norm = outT[:, 0:HD, :] / outT[:, HD:HD + 1, :]   # softmax denominator
        out[c] = norm.transpose(2, 0, 1).reshape(S, D)
    return out


if __name__ == "__main__":
    rng = np.random.default_rng(0)
    ins = {
        "query": rng.standard_normal((B, S, D)).astype(np.float32),
        "key": rng.standard_normal((B, S, D)).astype(np.float32),
        "value": rng.standard_normal((B, S, D)).astype(np.float32),
        "rel_pos_k": (rng.standard_normal((S, HD)) * 0.02).astype(np.float32),
        "rel_pos_v": (rng.standard_normal((S, HD)) * 0.02).astype(np.float32),
    }
    out = kernel(**ins)
    print("out", out.shape, out.dtype, np.abs(out).max())
